# revision 1
# baseline (speedup 1.0000x reference)
"""CapsuleLayer Trainium2 kernel.

Per-core work (data-parallel over batch N=8 -> 8 cores):
  x_i [t0=8, z0=32, 64, 64] -> conv(stride2,pad1,3x3, 512ch) -> u [t0, (z1,t1), 32x32]
  3 dynamic-routing iterations -> v [t1=8, z1=64, 32, 32]

Layout choices:
  - conv matmuls: lhsT = shifted x windows [K=97=(3kh*32z0 + ones), hw-chunk 128],
    rhs = host-repacked weights [97, 512=(z1*8+t1)], psum out [hw 128, 512].
    So u lands directly in "hw on partitions" routing layout, fp16 in SBUF.
  - routing einsums on DVE in fp16: multiplies via broadcast APs (2x mode),
    reductions via strided tree-adds.
  - softmax/maxpool on a transposed [64=(t0,t1), 1024=hw] view (PE transposes).
  - iteration-1 softmax is uniform => p1 = (sum_t0 u)/8 (no r needed).
"""

import numpy as np

import concourse.bass as bass
import concourse.tile as tile
from concourse import mybir
from concourse.bass_utils import run_bass_kernel_spmd

F32 = mybir.dt.float32
F32R = mybir.dt.float32r
F16 = mybir.dt.float16
BF16 = mybir.dt.bfloat16

T0, T1, Z1 = 8, 8, 64
H1 = W1 = 32
HW = H1 * W1          # 1024
NCHUNK = 8            # hw chunks of 128 partitions = 4 oh rows each
K = 97                # 3*32 + ones row
EPS = 1e-9
CLAMP = 60.0

AF = mybir.ActivationFunctionType
ALU = mybir.AluOpType

_MAXW = 1


def _split_waits(nc):
    """walrus in this container rejects >1 sync wait per instruction; hoist
    excess waits onto preceding NoOps on the same engine."""
    for fn in nc.m.functions:
        for blk in fn.blocks:
            new_insts = []
            for ins in blk.instructions:
                si = ins.sync_info
                if si is not None and len(si.on_wait) > _MAXW:
                    waits = list(si.on_wait)
                    extra, keep = waits[:-_MAXW], waits[-_MAXW:]
                    for i in range(0, len(extra), _MAXW):
                        new_insts.append(
                            mybir.InstNoOp(
                                name=f"{ins.name}-wsplit{i}",
                                engine=ins.engine,
                                sync_info=mybir.SyncInfo(
                                    on_wait=extra[i : i + _MAXW], on_update=[]
                                ),
                            )
                        )
                    si.on_wait = keep
                new_insts.append(ins)
            blk.instructions = new_insts
    return nc


def _squash_scale(nc, n2raw, sc, pre, post, tmp_pool, w):
    """Given n2raw[128,w] (= sum_z1 p^2 with p the UNSCALED accumulator),
    write sc[128,w] fp32 so that  v = squash(pre*p) = p * sc.
    sc = pre^3*n2raw / ((1 + pre^2*n2raw) * sqrt(pre^2*n2raw + EPS)) * post
    (post lets callers fold extra constant factors in)."""
    p2 = pre * pre
    t1 = tmp_pool.tile([128, w], F32, name="sqt1", tag="sqt1")
    # t1 = n2*pre^2 + 1
    nc.vector.tensor_scalar(t1[:], n2raw[:], p2, 1.0, op0=ALU.mult, op1=ALU.add)
    t2 = tmp_pool.tile([128, w], F32, name="sqt2", tag="sqt2")
    # t2 = n2*pre^2 + EPS
    nc.vector.tensor_scalar(t2[:], n2raw[:], p2, EPS, op0=ALU.mult, op1=ALU.add)
    t3 = tmp_pool.tile([128, w], F32, name="sqt3", tag="sqt3")
    nc.scalar.activation(t3[:], t2[:], AF.Sqrt)
    t4 = tmp_pool.tile([128, w], F32, name="sqt4", tag="sqt4")
    nc.vector.tensor_tensor(t4[:], t1[:], t3[:], op=ALU.mult)
    t5 = tmp_pool.tile([128, w], F32, name="sqt5", tag="sqt5")
    nc.vector.reciprocal(t5[:], t4[:])
    t6 = tmp_pool.tile([128, w], F32, name="sqt6", tag="sqt6")
    nc.vector.tensor_tensor(t6[:], t5[:], n2raw[:], op=ALU.mult)
    nc.vector.tensor_scalar_mul(sc[:], t6[:], p2 * pre * post)


def build_module(split=True, phases=99):
    nc = bass.Bass("TRN2", target_bir_lowering=False, debug=False)

    x = nc.dram_tensor("x", [T0, 32, 64, 64], F32, kind="ExternalInput")
    wt = nc.dram_tensor("wt", [3, K, 512], F32R, kind="ExternalInput")
    ident = nc.dram_tensor("ident", [128, 128], F16, kind="ExternalInput")
    ind8 = nc.dram_tensor("ind8", [64, T0], BF16, kind="ExternalInput")
    ind8t = nc.dram_tensor("ind8t", [T0, 64], F32, kind="ExternalInput")
    out = nc.dram_tensor("out", [T1, Z1, H1, W1], F32, kind="ExternalOutput")
    out_f = out.ap().rearrange("a b c d -> (a b c d)")

    with tile.TileContext(nc) as tc:
        import contextlib

        with contextlib.ExitStack() as ctx:
            consts = ctx.enter_context(tc.tile_pool(name="consts", bufs=1))
            ypool = ctx.enter_context(tc.tile_pool(name="ypool", bufs=1))
            upool = ctx.enter_context(tc.tile_pool(name="upool", bufs=1))
            tree = ctx.enter_context(tc.tile_pool(name="tree", bufs=3))
            small = ctx.enter_context(tc.tile_pool(name="small", bufs=3))
            persm = ctx.enter_context(tc.tile_pool(name="persm", bufs=1))
            rphase = ctx.enter_context(tc.tile_pool(name="rphase", bufs=1))
            vout = ctx.enter_context(tc.tile_pool(name="vout", bufs=1))
            ps_conv = ctx.enter_context(
                tc.tile_pool(name="ps_conv", bufs=2, space="PSUM")
            )
            ps_t = ctx.enter_context(tc.tile_pool(name="ps_t", bufs=2, space="PSUM"))
            ps_p1 = ctx.enter_context(tc.tile_pool(name="ps_p1", bufs=1, space="PSUM"))
            ps_s = ctx.enter_context(tc.tile_pool(name="ps_s", bufs=1, space="PSUM"))
            ps_sb = ctx.enter_context(tc.tile_pool(name="ps_sb", bufs=1, space="PSUM"))

            # ---------------- constants ----------------
            wt_sb = [
                consts.tile([K, 512], F32R, name=f"wt{kw}", tag=f"wt{kw}")
                for kw in range(3)
            ]
            for kw in range(3):
                nc.sync.dma_start(wt_sb[kw][:], wt.ap()[kw])
            id_sb = consts.tile([128, 128], F16, name="ident", tag="ident")
            nc.sync.dma_start(id_sb[:], ident.ap())
            ind8_sb = consts.tile([64, T0], BF16, name="ind8", tag="ind8")
            nc.sync.dma_start(ind8_sb[:], ind8.ap())
            ind8t_sb = consts.tile([T0, 64], F32, name="ind8t", tag="ind8t")
            nc.sync.dma_start(ind8t_sb[:], ind8t.ap())

            # ---------------- x staging -------
            # y_raw[(kh,z0), oh=32, c=66]; y_raw[p, oh, c] = x[z0, 2oh+kh-1, c-1]
            # y3[(kh,z0)+ones, kw-plane=3, oh=32, ow=32]:
            #   y3[p, kw, oh, ow] = x_pad[z0, 2oh+kh, 2ow+kw]  (single-stride
            #   windows so the matmul stationary operand has one free dim)
            # yraw2: x replicated to the 3 kh partition blocks (one fat DMA
            # with 16KB/partition descriptors -- the strided row/col selection
            # is done by on-chip engine copies instead of DMA descriptors).
            yraw2b = []
            for i in range(2):
                yr = ypool.tile([96, 64, 64], F32, name=f"yraw{i}", tag=f"yraw{i}")
                nc.vector.memset(yr[0:32, 0:1, :], 0.0)  # defined t0=0 garbage row
                yraw2b.append(yr)
            ybufs = []
            for i in range(2):
                y = ypool.tile([K, 3, 32, 32], F32R, name=f"y{i}", tag=f"y{i}")
                nc.vector.memset(y[96:97, :, :, :].bitcast(F32), 1.0)  # ones (bias)
                nc.vector.memset(y[0:96, 0, :, 0:1].bitcast(F32), 0.0)  # w=-1 pad
                nc.vector.memset(y[0:32, :, 0:1, :].bitcast(F32), 0.0)  # h=-1 pad
                ybufs.append(y)

            xa = x.ap()
            _cpeng = [nc.vector.tensor_copy, nc.scalar.copy, nc.gpsimd.tensor_copy]

            def load_y(t0):
                # Replicate x into 3 kh blocks PRE-SHIFTED by kh-1 rows, so
                # the deinterleave copies use uniform row APs on 96 partitions.
                # yraw2[(kh,z0), r, w] = x[t0, z0, r+kh-1, w].
                yraw2 = yraw2b[t0 % 2]
                if t0 == 0:
                    nc.sync.dma_start(yraw2[0:32, 1:64, :], xa[0, :, 0:63, :])
                    src = bass.AP(
                        tensor=xa.tensor,
                        offset=0,
                        ap=[[64, 2], [4096, 32], [1, 4032]],
                    )
                    nc.sync.dma_start(
                        yraw2[32:96, 0:63, :].rearrange("p a b -> p (a b)"), src
                    )
                else:
                    src = bass.AP(
                        tensor=xa.tensor,
                        offset=t0 * 32 * 4096 - 64,
                        ap=[[64, 3], [4096, 32], [1, 4032]],
                    )
                    nc.sync.dma_start(
                        yraw2[:, 0:63, :].rearrange("p a b -> p (a b)"), src
                    )
                y = ybufs[t0 % len(ybufs)]
                # 3 plane copies (rows 2oh uniform, cols 2ow+pl-1); then re-zero
                # the h=-1 pad row (kh=0 block oh=0) the copies clobbered.
                for pl in range(3):
                    ow0, col0, n_ow = (1, 1, 31) if pl == 0 else (0, pl - 1, 32)
                    src_v = yraw2[0:96, 0:63:2, col0 : col0 + 2 * n_ow - 1 : 2]
                    dst_v = y[0:96, pl, 0:32, ow0 : ow0 + n_ow]
                    _cpeng[pl](dst_v, src_v)
                nc.gpsimd.memset(y[0:32, :, 0:1, :].bitcast(F32), 0.0)
                return y

            # persistent routing state
            b = [persm.tile([128, T0 * T1], F16, name=f"b{c}", tag=f"b{c}") for c in range(NCHUNK)]
            r = [persm.tile([128, T0, T1], F16, name=f"r{c}", tag=f"r{c}") for c in range(NCHUNK)]
            # small staging for transposed output blocks (DMA'd out per (c,j))

            def t0_sum_pe(src, p_sb, pool=None, tag="pu"):
                """p_sb[128,512] f16 <- sum_t0 src[128, T0, 512] via identity
                matmuls accumulating in PSUM (PE is idle during routing)."""
                pp = (pool or ps_conv).tile([128, 512], F32, name=tag, tag=tag)
                for t0 in range(T0):
                    nc.tensor.matmul(
                        pp[:],
                        id_sb[:],
                        src[:, t0, :],
                        start=(t0 == 0),
                        stop=(t0 == T0 - 1),
                    )
                nc.scalar.copy(p_sb[:], pp[:])
                return p_sb

            def z1_sum(c, prod, first):
                """b[c] (+)= sum_z1 prod[128, T0, Z1, T1]: two DVE tree levels,
                then 16 identity-matmuls accumulate the tail on PE."""
                l1 = tree.tile([128, T0, 32, T1], F16, name="zt0", tag="zt0", bufs=2)
                s = prod.rearrange("p a (zh two) b -> p a zh two b", two=2)
                nc.vector.tensor_tensor(
                    l1[:], s[:, :, :, 0, :], s[:, :, :, 1, :], op=ALU.add
                )
                pz = ps_conv.tile([128, T0 * T1], F32, name="pu", tag="pu")
                for z in range(32):
                    nc.tensor.matmul(
                        pz.rearrange("p (a b) -> p a b", b=T1),
                        id_sb[:],
                        l1[:, :, z, :],
                        start=(z == 0),
                        stop=(z == 31),
                    )
                if first:
                    nc.scalar.copy(b[c][:], pz[:])
                else:
                    nc.vector.tensor_tensor(b[c][:], b[c][:], pz[:], op=ALU.add)

            def squash_one(c, p_acc, pre, v_tile):
                n2 = small.tile([128, T1], F32, name="sqn2", tag="sqn2", bufs=4)
                squash_n2(c, p_acc, n2, single=True)
                sc = small.tile([128, T1], F32, name="sqsc", tag="sqsc", bufs=4)
                _squash_scale(nc, n2, sc, pre, 1.0, small, T1)
                squash_v(c, p_acc, sc, v_tile, single=True)

            def squash_n2(c, p_acc, n2_all, single=False):
                """n2_all[:, 8c:8c+8] <- sum_z1 p^2 per t1."""
                pz = p_acc.rearrange("p (z t) -> p z t", t=T1)
                sq = small.tile([128, Z1, T1], F16, name="sqsq", tag="sqsq", bufs=4)
                nc.gpsimd.tensor_tensor(sq[:], pz[:], pz[:], op=ALU.mult)
                dst = n2_all[:] if single else n2_all[:, T1 * c : T1 * (c + 1)]
                nc.vector.tensor_reduce(
                    dst,
                    sq.transpose([0, 2, 1]),
                    axis=mybir.AxisListType.X,
                    op=ALU.add,
                )

            def squash_v(c, p_acc, sc_all, v_tile, single=False):
                pz = p_acc.rearrange("p (z t) -> p z t", t=T1)
                scs = sc_all[:] if single else sc_all[:, T1 * c : T1 * (c + 1)]
                scb = scs.unsqueeze(1).broadcast_to([128, Z1, T1])
                eng = nc.gpsimd
                eng.tensor_tensor(
                    v_tile.rearrange("p (z t) -> p z t", t=T1), pz[:], scb, op=ALU.mult
                )

            def b_einsum(c, v_tile, first):
                """b[c] (+)= sum_z1 u * v  (v broadcast over t0)."""
                prod = tree.tile([128, T0, Z1 * T1], F16, name="trP", tag="trP", bufs=2)
                vb = (
                    v_tile.unsqueeze(1).broadcast_to([128, T0, Z1 * T1])
                )
                nc.vector.tensor_tensor(prod[:], U[c][:], vb, op=ALU.mult)
                z1_sum(c, prod.rearrange("p a (z b) -> p a z b", b=T1), first)

            def r_phase():
                """r[c] <- softmax_t1(maxpool3x3(b)) for all chunks."""
                bT = rphase.tile([64, H1, W1], F16, name="bT", tag="bT")
                for c in range(NCHUNK):
                    pt = ps_t.tile([128, 128], F16, name="ptr", tag="ptr")
                    nc.tensor.transpose(pt[0:64, :], b[c][:], id_sb[:])
                    nc.scalar.copy(
                        bT[:, 4 * c : 4 * c + 4, :].rearrange("p a b -> p (a b)"),
                        pt[0:64, :],
                    )
                # maxpool: w direction then h direction
                mw = rphase.tile([64, H1, W1], F16, name="mw", tag="mw", bufs=1)
                nc.vector.tensor_tensor(
                    mw[:, :, 0:31], bT[:, :, 0:31], bT[:, :, 1:32], op=ALU.max
                )
                nc.gpsimd.tensor_copy(mw[:, :, 31:32], bT[:, :, 31:32])
                nc.vector.tensor_tensor(
                    mw[:, :, 1:32], mw[:, :, 1:32], bT[:, :, 0:31], op=ALU.max
                )
                mp = rphase.tile([64, H1, W1], F16, name="mp", tag="mp", bufs=1)
                nc.vector.tensor_tensor(
                    mp[:, 0:31, :], mw[:, 0:31, :], mw[:, 1:32, :], op=ALU.max
                )
                nc.gpsimd.tensor_copy(mp[:, 31:32, :], mw[:, 31:32, :])
                nc.vector.tensor_tensor(
                    mp[:, 1:32, :], mp[:, 1:32, :], mw[:, 0:31, :], op=ALU.max
                )
                nc.vector.tensor_scalar_min(mp[:], mp[:], CLAMP)
                # E = exp(mp)  (bf16: range safety)
                E = rphase.tile([64, HW], BF16, name="E", tag="E")
                nc.scalar.activation(
                    E.rearrange("p (a b) -> p a b", b=W1), mp[:], AF.Exp
                )
                # S = sum_t1 E : [8, 1024] psum
                S = ps_s.tile([T0, HW], F32, name="S", tag="S")
                for h in range(2):
                    nc.tensor.matmul(
                        S[:, 512 * h : 512 * (h + 1)],
                        ind8_sb[:],
                        E[:, 512 * h : 512 * (h + 1)],
                        start=True,
                        stop=True,
                    )
                Sr = rphase.tile([T0, HW], F32, name="Sr", tag="Sr", bufs=1)
                nc.vector.reciprocal(Sr[:], S[:])
                rT = rphase.tile([64, HW], F16, name="rT", tag="rT")
                for h in range(2):
                    Sb = ps_sb.tile([64, 512], F32, name="Sb", tag="Sb")
                    nc.tensor.matmul(
                        Sb[:],
                        ind8t_sb[:],
                        Sr[:, 512 * h : 512 * (h + 1)],
                        start=True,
                        stop=True,
                    )
                    nc.vector.tensor_tensor(
                        rT[:, 512 * h : 512 * (h + 1)],
                        E[:, 512 * h : 512 * (h + 1)],
                        Sb[:],
                        op=ALU.mult,
                    )
                # transpose back per chunk -> r[c] [128, (t0,t1)]
                rTv = rT.rearrange("p (a b) -> p a b", b=W1)
                for c in range(NCHUNK):
                    pt = ps_t.tile([128, 128], F16, name="ptr", tag="ptr")
                    nc.tensor.transpose(
                        pt[:, 0:64],
                        rTv[:, 4 * c : 4 * c + 4, :].rearrange("p a b -> p (a b)"),
                        id_sb[0:64, 0:64],
                    )
                    nc.scalar.copy(r[c].rearrange("p a b -> p (a b)"), pt[:, 0:64])

            def p_einsum(c, p_tile):
                """p = sum_t0 r * u  : [128, 512] f16."""
                prod = tree.tile([128, T0, 512], F16, name="trP", tag="trP", bufs=2)
                rb = (
                    r[c]
                    .unsqueeze(2)
                    .broadcast_to([128, T0, Z1, T1])
                )
                nc.vector.tensor_tensor(
                    prod.rearrange("p a (z b) -> p a z b", b=T1), U[c][:], rb,
                    op=ALU.mult,
                )
                t0_sum_pe(prod[:], p_tile)

            vT = [
                vout.tile([128, HW], F32, name=f"vT{j}", tag=f"vT{j}")
                for j in range(4)
            ]

            def out_chunk(c, v_tile):
                """transpose v [128hw, 512] -> vT[j][:, 128c:] fp32 staging."""
                for j in range(4):
                    pt = ps_t.tile([128, 128], F16, name="ptr", tag="ptr")
                    nc.tensor.transpose(
                        pt[:], v_tile[:, 128 * j : 128 * (j + 1)], id_sb[:]
                    )
                    nc.scalar.copy(vT[j][:, 128 * c : 128 * (c + 1)], pt[:])

            # ---------------- conv ----------------
            U = [upool.tile([128, T0, 512], F16, name=f"U{c}", tag=f"U{c}") for c in range(NCHUNK)]
            ps1 = [
                small.tile([128, 512], F16, name=f"pacc{c}", tag=f"pacc{c}", bufs=1)
                for c in range(NCHUNK)
            ]
            for t0 in range(T0):
                y = load_y(t0)
                for c in range(NCHUNK):
                    pu = ps_conv.tile([128, 512], F32, name="pu", tag="pu")
                    yv = y.rearrange("p k a b -> p k (a b)")
                    for kw in range(3):
                        nc.tensor.matmul(
                            pu[:],
                            yv[:, kw, 128 * c : 128 * (c + 1)],
                            wt_sb[kw][:],
                            start=(kw == 0),
                            stop=(kw == 2),
                        )
                    if (t0 + c) % 2 == 0:
                        nc.scalar.copy(U[c][:, t0, :], pu[:])
                    else:
                        nc.vector.tensor_copy(U[c][:, t0, :], pu[:])
                    if t0 == T0 - 1 and phases >= 1:
                        t0_sum_pe(U[c][:], ps1[c], pool=ps_p1, tag="pp1")

            # ---------------- routing ----------------
            # Stage-batched loops: all chunks' stage-k ops are emitted together
            # so each engine always has independent work from other chunks.
            # iter 1: r uniform=1/8 -> p1 = (sum_t0 u)/8; v1 = squash(p1)
            if phases >= 1:
                vs = [
                    small.tile([128, 512], F16, name=f"vt{c}", tag=f"vt{c}", bufs=1)
                    for c in range(NCHUNK)
                ]
                for c in range(NCHUNK):
                    squash_one(c, ps1[c], 1.0 / T0, vs[c])
                for c in range(NCHUNK):
                    b_einsum(c, vs[c], first=True)

            # iters 2..3
            for it in (2, 3):
                if it > phases:
                    break
                r_phase()
                last = it == 3
                ps = [
                    small.tile([128, 512], F16, name=f"pacc{c}", tag=f"pacc{c}", bufs=1)
                    for c in range(NCHUNK)
                ]
                vs = [
                    small.tile([128, 512], F16, name=f"vt{c}", tag=f"vt{c}", bufs=1)
                    for c in range(NCHUNK)
                ]
                for c in range(NCHUNK):
                    p_einsum(c, ps[c])
                for c in range(NCHUNK):
                    squash_one(c, ps[c], 1.0, vs[c])
                for c in range(NCHUNK):
                    if not last:
                        b_einsum(c, vs[c], first=False)
                    else:
                        out_chunk(c, vs[c])

            # vT[j] partitions p=(z1l*8+t1), z1=j*16+z1l -> out row t1*64+z1
            for j in range(4 if phases >= 3 else 0):
                dst = bass.AP(
                    tensor=out_f.tensor,
                    offset=j * 16 * HW,
                    ap=[[HW, 16], [64 * HW, 8], [1, HW]],
                )
                nc.sync.dma_start(dst, vT[j][:])

    return _split_waits(nc) if split else nc


# ---------------------------------------------------------------------------
_NC = None


def _get_nc(split=True):
    global _NC
    if _NC is None:
        _NC = build_module(split)
    return _NC


def _host_prep(W, bias):
    # wt[kw][kh*32+z0, z1*8+t1] = W[t1*64+z1, z0, kh, kw]; bias in wt[1][96]
    Wr = np.asarray(W, np.float32).reshape(T1, Z1, 32, 3, 3)
    wt = np.zeros((3, K, 512), np.float32)
    # -> [kw, kh, z0, z1, t1]
    Wp = np.transpose(Wr, (4, 3, 2, 1, 0))
    wt[:, :96, :] = Wp.reshape(3, 96, 512)
    bz = np.asarray(bias, np.float32).reshape(T1, Z1).T.reshape(512)  # z1*8+t1
    wt[1, 96, :] = bz
    return wt


def _consts():
    ident = np.eye(128, dtype=np.float16)
    ind8 = np.zeros((64, T0), np.float32)
    for p in range(64):
        ind8[p, p // T1] = 1.0
    ind8t = np.zeros((T0, 64), np.float32)
    for m in range(64):
        ind8t[m // T1, m] = 1.0
    import ml_dtypes

    return ident, ind8.astype(ml_dtypes.bfloat16), ind8t.astype(np.float32)


def _run(inputs, trace=False, **kw):
    x = np.ascontiguousarray(np.asarray(inputs["x"], np.float32))
    wt = _host_prep(inputs["W"], inputs["bias"])
    ident, ind8, ind8t = _consts()
    nc = _get_nc()
    in_maps = [
        {"x": x[i], "wt": wt, "ident": ident, "ind8": ind8, "ind8t": ind8t}
        for i in range(8)
    ]
    res = run_bass_kernel_spmd(nc, in_maps, core_ids=list(range(8)), trace=trace, **kw)
    full = np.stack([res.results[i]["out"] for i in range(8)], axis=0)
    return full, res


def kernel(**inputs):
    full, _ = _run(inputs)
    return full


if __name__ == "__main__":
    rng = np.random.default_rng(0)
    ins = {
        "x": rng.normal(size=(8, 8, 32, 64, 64)).astype(np.float32),
        "W": (rng.normal(size=(512, 32, 3, 3)) * 0.05).astype(np.float32),
        "bias": (rng.normal(size=(512,)) * 0.01).astype(np.float32),
    }
    out = kernel(**ins)
    print(out.shape, out.dtype)



# revision 25
# speedup vs baseline: 1.0866x; 1.0866x over previous
"""CapsuleLayer Trainium2 kernel (v2).

Per-core work (data-parallel over batch N=8 -> 8 cores):
  x_i [t0=8, z0=32, 64, 64] -> conv(stride2,pad1,3x3, 512ch) -> u [t0, (z1,t1), 32x32]
  3 dynamic-routing iterations -> v [t1=8, z1=64, 32, 32]

v2 layout/schedule changes vs v1:
  - U in one tensor [128, c, t0, 512] fp16; conv psum pairs (2 chunks/bank-pair)
    copied out in single [128,1024] ops.
  - p1 = sum_t0 u computed by incremental DVE pair-adds DURING conv
    (PE identity-sum dropped).
  - z1-sums (b updates) fully on PE, accumulating into a persistent PSUM
    bank (memset once, start=False) -> no DVE tree, no b-add, b stays put.
  - n2 (squash norm) via tiny PE identity matmuls into a shared psum tile;
    squash scale chain batched across all 8 chunks (5 ops/iter, not 5*8).
  - output: per-chunk f32 transposes -> SBUF staging -> per-(c,j) DMAs.
  - engine assignment tuned: einsum mults split DVE/Pool, squash sq on
    Pool, v-mults DVE, psum copies on Act.
"""

import numpy as np

import concourse.bass as bass
import concourse.tile as tile
from concourse import mybir
from concourse.bass_utils import run_bass_kernel_spmd

F32 = mybir.dt.float32
F32R = mybir.dt.float32r
F16 = mybir.dt.float16
BF16 = mybir.dt.bfloat16

T0, T1, Z1 = 8, 8, 64
H1 = W1 = 32
HW = H1 * W1          # 1024
NCHUNK = 8            # hw chunks of 128 partitions = 4 oh rows each
K = 97                # 3*32 + ones row
EPS = 1e-9
CLAMP = 60.0

AF = mybir.ActivationFunctionType
ALU = mybir.AluOpType

_MAXW = 1

# engine-assignment knobs (swept offline via TimelineSim)
CONFIG = dict(
    # einsum-mult chunks sent to gpsimd (Pool) instead of DVE, per site
    # sites: 0=b1, 1=p2, 2=b2, 3=p3
    pool_mult={0: (), 1: (), 2: (), 3: ()},
    # einsum mults: split along t0, first part DVE, rest Pool (8,0)=all DVE
    mult_split=(6, 2),
    # z1-sum: pair-tree level engine per chunk (None = all 64 on PE)
    z1_tree=None,
    # conv pair-copy engine cycle (per pair index)
    conv_copy=("act",),
    # squash: sq engine cycle, split chunks into halves
    sq_eng=("dve", "pool"),
    squash_halves=True,
    rt_dma=False,
)


def _split_waits(nc):
    """walrus in this container rejects >1 sync wait per instruction; hoist
    excess waits onto preceding NoOps on the same engine."""
    for fn in nc.m.functions:
        for blk in fn.blocks:
            new_insts = []
            for ins in blk.instructions:
                si = ins.sync_info
                if si is not None and len(si.on_wait) > _MAXW:
                    waits = list(si.on_wait)
                    extra, keep = waits[:-_MAXW], waits[-_MAXW:]
                    for i in range(0, len(extra), _MAXW):
                        new_insts.append(
                            mybir.InstNoOp(
                                name=f"{ins.name}-wsplit{i}",
                                engine=ins.engine,
                                sync_info=mybir.SyncInfo(
                                    on_wait=extra[i : i + _MAXW], on_update=[]
                                ),
                            )
                        )
                    si.on_wait = keep
                new_insts.append(ins)
            blk.instructions = new_insts
    return nc


def build_module(split=True, phases=99):
    nc = bass.Bass("TRN2", target_bir_lowering=False, debug=False)

    x = nc.dram_tensor("x", [T0, 32, 64, 64], F32, kind="ExternalInput")
    wt = nc.dram_tensor("wt", [3, K, 512], F32R, kind="ExternalInput")
    ident = nc.dram_tensor("ident", [128, 128], F16, kind="ExternalInput")
    identf = nc.dram_tensor("identf", [128, 128], F32, kind="ExternalInput")
    ind8 = nc.dram_tensor("ind8", [64, T0], BF16, kind="ExternalInput")
    ind8t = nc.dram_tensor("ind8t", [T0, 64], BF16, kind="ExternalInput")
    out = nc.dram_tensor("out", [T1, Z1, H1, W1], F32, kind="ExternalOutput")
    out_f = out.ap().rearrange("a b c d -> (a b c d)")

    with tile.TileContext(nc) as tc:
        import contextlib

        with contextlib.ExitStack() as ctx:
            consts = ctx.enter_context(tc.tile_pool(name="consts", bufs=1))
            ypool = ctx.enter_context(tc.tile_pool(name="ypool", bufs=1))
            upool = ctx.enter_context(tc.tile_pool(name="upool", bufs=1))
            small = ctx.enter_context(tc.tile_pool(name="small", bufs=3))
            prodp = ctx.enter_context(tc.tile_pool(name="prodp", bufs=2))
            persm = ctx.enter_context(tc.tile_pool(name="persm", bufs=1))
            rphase = ctx.enter_context(tc.tile_pool(name="rphase", bufs=1))
            vout = ctx.enter_context(tc.tile_pool(name="vout", bufs=1))
            ps_acc = ctx.enter_context(tc.tile_pool(name="ps_acc", bufs=1, space="PSUM"))
            ps_rt_box = {}

            def ps_rt():
                return ps_rt_box["pool"]

            # ---------------- x staging (as v1) ----------------
            yraw2b = []
            for i in range(2):
                yr = ypool.tile([96, 64, 64], F32, name=f"yraw{i}", tag=f"yraw{i}")
                nc.vector.memset(yr[0:32, 0:1, :], 0.0)  # defined t0=0 garbage row
                yraw2b.append(yr)
            ybufs = []
            for i in range(2):
                y = ypool.tile([K, 3, 32, 32], F32R, name=f"y{i}", tag=f"y{i}")
                nc.vector.memset(y[96:97, :, :, :].bitcast(F32), 1.0)  # ones (bias)
                nc.vector.memset(y[0:96, 0, :, 0:1].bitcast(F32), 0.0)  # w=-1 pad
                nc.vector.memset(y[0:32, :, 0:1, :].bitcast(F32), 0.0)  # h=-1 pad
                ybufs.append(y)

            xa = x.ap()
            _cpeng = [nc.vector.tensor_copy, nc.scalar.copy, nc.gpsimd.tensor_copy]

            def load_yraw(t0):
                # Replicate x into 3 kh blocks PRE-SHIFTED by kh-1 rows, so
                # the deinterleave copies use uniform row APs on 96 partitions.
                # yraw2[(kh,z0), r, w] = x[t0, z0, r+kh-1, w].
                yraw2 = yraw2b[t0 % 2]
                if t0 == 0:
                    nc.sync.dma_start(yraw2[0:32, 1:64, :], xa[0, :, 0:63, :])
                    src = bass.AP(
                        tensor=xa.tensor,
                        offset=0,
                        ap=[[64, 2], [4096, 32], [1, 4032]],
                    )
                    nc.sync.dma_start(
                        yraw2[32:96, 0:63, :].rearrange("p a b -> p (a b)"), src
                    )
                else:
                    src = bass.AP(
                        tensor=xa.tensor,
                        offset=t0 * 32 * 4096 - 64,
                        ap=[[64, 3], [4096, 32], [1, 4032]],
                    )
                    nc.sync.dma_start(
                        yraw2[:, 0:63, :].rearrange("p a b -> p (a b)"), src
                    )

            def build_y(t0):
                yraw2 = yraw2b[t0 % 2]
                y = ybufs[t0 % len(ybufs)]
                # 3 plane copies (rows 2oh uniform, cols 2ow+pl-1); then re-zero
                # the h=-1 pad row (kh=0 block oh=0) the copies clobbered.
                for pl in range(3):
                    ow0, col0, n_ow = (1, 1, 31) if pl == 0 else (0, pl - 1, 32)
                    src_v = yraw2[0:96, 0:63:2, col0 : col0 + 2 * n_ow - 1 : 2]
                    dst_v = y[0:96, pl, 0:32, ow0 : ow0 + n_ow]
                    _cpeng[pl](dst_v, src_v)
                nc.gpsimd.memset(y[0:32, :, 0:1, :].bitcast(F32), 0.0)
                return y

            load_yraw(0)

            # ---------------- constants (DMAs issued after first x load) ----
            wt_sb = [
                consts.tile([K, 512], F32R, name=f"wt{kw}", tag=f"wt{kw}")
                for kw in range(3)
            ]
            for kw in range(3):
                nc.sync.dma_start(wt_sb[kw][:], wt.ap()[kw])
            id_sb = consts.tile([128, 128], F16, name="ident", tag="ident")
            nc.sync.dma_start(id_sb[:], ident.ap())
            idf_sb = consts.tile([128, 128], F32, name="identf", tag="identf")
            nc.sync.dma_start(idf_sb[:], identf.ap())
            ind8_sb = consts.tile([64, T0], BF16, name="ind8", tag="ind8")
            nc.sync.dma_start(ind8_sb[:], ind8.ap())
            ind8t_sb = consts.tile([T0, 64], BF16, name="ind8t", tag="ind8t")
            nc.sync.dma_start(ind8t_sb[:], ind8t.ap())

            # ---------------- persistent PSUM accumulators ----------------
            # b accumulates across iters via start=False matmuls; memset once.
            bps = ps_acc.tile([128, NCHUNK * T0 * T1], F32, name="bps", tag="bps")
            nc.vector.memset(bps[:], 0.0)
            # n2 (squash norms) all chunks: re-memset each iter
            n2ps = ps_acc.tile([128, NCHUNK * T1], F32, name="n2ps", tag="n2ps")

            # U_all[:, c, t0, :] fp16
            U = upool.tile([128, NCHUNK, T0, 512], F16, name="U", tag="U")
            # p1 running sums (accumulated during conv on DVE)
            p1s = [
                upool.tile([128, 512], F16, name=f"p1_{c}", tag=f"p1_{c}")
                for c in range(NCHUNK)
            ]

            def bslice(c):
                return bps[:, 64 * c : 64 * (c + 1)].rearrange(
                    "p (a b) -> p a b", b=T1
                )

            def z1_sum_pe(c, prod):
                """bps[c] += sum_z1 prod[128, T0, Z1, T1]: optional pair-tree
                level on DVE/Pool, then accumulating identity matmuls into the
                PSUM-resident b (never reset)."""
                dst = bslice(c)
                zt = CONFIG["z1_tree"]
                te = None if zt is None else zt[c % len(zt)]
                if te is None:
                    for z in range(Z1):
                        nc.tensor.matmul(
                            dst,
                            id_sb[:],
                            prod[:, :, z, :],
                            start=False,
                            stop=(z == Z1 - 1),
                            skip_group_check=True,
                        )
                    return
                l1t = prodp.tile(
                    [128, T0, 32, T1], F16, name="zt", tag=f"zt{c % 2}", bufs=1
                )
                sv = prod.rearrange("p a (zh two) b -> p a zh two b", two=2)
                eng = (
                    nc.vector.tensor_tensor if te == "dve" else nc.gpsimd.tensor_tensor
                )
                eng(l1t[:], sv[:, :, :, 0, :], sv[:, :, :, 1, :], op=ALU.add)
                for z in range(32):
                    nc.tensor.matmul(
                        dst,
                        id_sb[:],
                        l1t[:, :, z, :],
                        start=False,
                        stop=(z == 31),
                        skip_group_check=True,
                    )

            def n2_pe(c, sq):
                """n2ps[:, 8c:8c+8] += sum_z1 sq[128, Z1, T1] (psum-resident,
                memset per iter)."""
                dst = n2ps[:, T1 * c : T1 * (c + 1)]
                for z in range(Z1):
                    nc.tensor.matmul(
                        dst,
                        id_sb[:],
                        sq[:, z, :],
                        start=False,
                        stop=(z == Z1 - 1),
                        skip_group_check=True,
                    )

            def sc_chain(pre, sc_all, n2v, tag, w):
                """sc_all[128, 64] f32 <- batched squash scale for all chunks.
                v = squash(pre*p) = p * sc;  n2raw = n2ps (sum p^2, unscaled).
                sc = C*n2/((1+p2*n2)*sqrt(p2*n2+eps)), C = pre^3 (post folded
                by caller via pre).  Computed as  1/( ((n2*p2+1)/C) * sqrt() )*n2."""
                p2 = pre * pre
                C = p2 * pre
                t1 = small.tile([128, w], F32, name=f"sct1{tag}", tag=f"sct1{tag}", bufs=1)
                nc.vector.tensor_scalar(
                    t1[:], n2v, p2 / C, 1.0 / C, op0=ALU.mult, op1=ALU.add
                )
                t2 = small.tile([128, w], F32, name=f"sct2{tag}", tag=f"sct2{tag}", bufs=1)
                nc.vector.tensor_scalar(
                    t2[:], n2v, p2, EPS, op0=ALU.mult, op1=ALU.add
                )
                t3 = small.tile([128, w], F32, name=f"sct3{tag}", tag=f"sct3{tag}", bufs=1)
                nc.scalar.activation(t3[:], t2[:], AF.Sqrt)
                t4 = small.tile([128, w], F32, name=f"sct4{tag}", tag=f"sct4{tag}", bufs=1)
                nc.vector.tensor_tensor(t4[:], t1[:], t3[:], op=ALU.mult)
                t5 = small.tile([128, w], F32, name=f"sct5{tag}", tag=f"sct5{tag}", bufs=1)
                nc.vector.reciprocal(t5[:], t4[:])
                nc.vector.tensor_tensor(sc_all[:], t5[:], n2v, op=ALU.mult)

            # persistent r tiles
            r = [
                persm.tile([128, T0, T1], F16, name=f"r{c}", tag=f"r{c}")
                for c in range(NCHUNK)
            ]

            def r_phase():
                """r[c] <- softmax_t1(maxpool3x3(b)) for all chunks; b read
                from PSUM."""
                # bsb = min(b, CLAMP)  (clamp folded here; commutes with maxpool)
                bsb = rphase.tile([128, NCHUNK, T0 * T1], F16, name="bsb", tag="bsb")
                for c in range(NCHUNK):
                    nc.vector.tensor_scalar_min(
                        bsb[:, c, :], bps[:, 64 * c : 64 * (c + 1)], CLAMP
                    )
                bT = rphase.tile([64, H1, W1], F16, name="bT", tag="bT", bufs=1)
                for c in range(NCHUNK):
                    pt = ps_rt().tile([128, 128], F32, name="pt", tag="pt", bufs=2)[
                        :
                    ].bitcast(F16)[:, 0:128]
                    nc.tensor.transpose(pt[0:64, :], bsb[:, c, :], id_sb[:])
                    (nc.scalar.copy if c % 2 == 0 else nc.vector.tensor_copy)(
                        bT[:, 4 * c : 4 * c + 4, :].rearrange("p a b -> p (a b)"),
                        pt[0:64, :],
                    )
                # maxpool: w direction then h direction
                mw = rphase.tile([64, H1, W1], F16, name="mw", tag="mw", bufs=1)
                nc.vector.tensor_tensor(
                    mw[:, :, 0:31], bT[:, :, 0:31], bT[:, :, 1:32], op=ALU.max
                )
                nc.gpsimd.tensor_copy(mw[:, :, 31:32], bT[:, :, 31:32])
                nc.vector.tensor_tensor(
                    mw[:, :, 1:32], mw[:, :, 1:32], bT[:, :, 0:31], op=ALU.max
                )
                mp = rphase.tile([64, H1, W1], F16, name="mp", tag="bT", bufs=1)
                nc.vector.tensor_tensor(
                    mp[:, 0:31, :], mw[:, 0:31, :], mw[:, 1:32, :], op=ALU.max
                )
                nc.gpsimd.tensor_copy(mp[:, 31:32, :], mw[:, 31:32, :])
                nc.vector.tensor_tensor(
                    mp[:, 1:32, :], mp[:, 1:32, :], mw[:, 0:31, :], op=ALU.max
                )
                # E = exp(mp)  (bf16: range safety; clamp folded into bsb)
                E = rphase.tile([64, HW], BF16, name="E", tag="mw", bufs=1)
                Ev = E.rearrange("p (a b) -> p a b", b=W1)
                for h in range(2):
                    nc.scalar.activation(
                        Ev[:, 16 * h : 16 * (h + 1), :], mp[:, 16 * h : 16 * (h + 1), :],
                        AF.Exp,
                    )
                # S = sum_t1 E : [8, 1024] psum
                Sr = rphase.tile([T0, HW], BF16, name="Sr", tag="Sr", bufs=1)
                for h in range(2):
                    S = ps_rt().tile([T0, 512], F32, name="S", tag="S")
                    nc.tensor.matmul(
                        S[:],
                        ind8_sb[:],
                        E[:, 512 * h : 512 * (h + 1)],
                        start=True,
                        stop=True,
                    )
                    with nc.allow_low_precision(reason="softmax denom bf16 ok"):
                        nc.vector.reciprocal(Sr[:, 512 * h : 512 * (h + 1)], S[:])
                rT = rphase.tile([64, HW], F16, name="rT", tag="rT")
                for h in range(2):
                    Sb = ps_rt().tile([64, 512], F32, name="Sb", tag="Sb")
                    nc.tensor.matmul(
                        Sb[:],
                        ind8t_sb[:],
                        Sr[:, 512 * h : 512 * (h + 1)],
                        start=True,
                        stop=True,
                    )
                    nc.vector.tensor_tensor(
                        rT[:, 512 * h : 512 * (h + 1)],
                        E[:, 512 * h : 512 * (h + 1)],
                        Sb[:],
                        op=ALU.mult,
                    )
                # transpose back per chunk -> r[c] [128, (t0,t1)]
                rTv = rT.rearrange("p (a b) -> p a b", b=W1)
                for c in range(NCHUNK):
                    if CONFIG["rt_dma"]:
                        nc.sync.dma_start_transpose(
                            r[c].rearrange("p a b -> p (a b)"),
                            rTv[:, 4 * c : 4 * c + 4, :].rearrange("p a b -> p (a b)"),
                        )
                        continue
                    pt = ps_rt().tile([128, 128], F32, name="pt", tag="pt", bufs=2)[
                        :
                    ].bitcast(F16)[:, 0:128]
                    nc.tensor.transpose(
                        pt[:, 0:64],
                        rTv[:, 4 * c : 4 * c + 4, :].rearrange("p a b -> p (a b)"),
                        id_sb[0:64, 0:64],
                    )
                    (nc.scalar.copy if c % 2 == 0 else nc.vector.tensor_copy)(
                        r[c].rearrange("p a b -> p (a b)"), pt[:, 0:64]
                    )

            def squash_phase(ps_list, pre, it, consume, vdt=F16):
                """v[c] = squash(pre * p_c); calls consume(c, v) as each v is
                emitted. Chunks processed in half-batches so the scale chain
                barrier only spans 4 chunks."""
                nc.vector.memset(n2ps[:], 0.0)
                halves = (
                    ((0, 1, 2, 3), (4, 5, 6, 7))
                    if CONFIG["squash_halves"]
                    else (tuple(range(NCHUNK)),)
                )
                sqe = CONFIG["sq_eng"]
                for hi, half in enumerate(halves):
                    for c in half:
                        sq = small.tile(
                            [128, Z1, T1], F16, name=f"sq{it}_{c}",
                            tag=f"sq{c % 3}", bufs=1,
                        )
                        pz = ps_list[c].rearrange("p (z t) -> p z t", t=T1)
                        eng = (
                            nc.vector.tensor_tensor
                            if sqe[c % len(sqe)] == "dve"
                            else nc.gpsimd.tensor_tensor
                        )
                        eng(sq[:], pz[:], pz[:], op=ALU.mult)
                        n2_pe(c, sq)
                    w = len(half) * T1
                    lo = half[0] * T1
                    sc_all = small.tile(
                        [128, w], F32, name=f"sc{it}_{hi}", tag=f"sc_all{hi}", bufs=1
                    )
                    sc_chain(pre, sc_all, n2ps[:, lo : lo + w], f"{it}_{hi}", w)
                    for ci, c in enumerate(half):
                        v = small.tile(
                            [128, 512], vdt, name=f"v{it}_{c}", tag=f"vs{c}", bufs=1
                        )
                        scb = (
                            sc_all[:, T1 * ci : T1 * (ci + 1)]
                            .unsqueeze(1)
                            .broadcast_to([128, Z1, T1])
                        )
                        nc.vector.tensor_tensor(
                            v.rearrange("p (z t) -> p z t", t=T1),
                            ps_list[c].rearrange("p (z t) -> p z t", t=T1),
                            scb,
                            op=ALU.mult,
                        )
                        consume(c, v)

            def split_mult(prod4, u4, rhs4):
                """prod4 = u4 * rhs4 on [128, T0, Z1, T1] views, t0-split
                across DVE and Pool per CONFIG[mult_split]."""
                nd, npl = CONFIG["mult_split"]
                if nd:
                    nc.vector.tensor_tensor(
                        prod4[:, 0:nd], u4[:, 0:nd], rhs4[:, 0:nd], op=ALU.mult
                    )
                if npl:
                    nc.gpsimd.tensor_tensor(
                        prod4[:, nd:], u4[:, nd:], rhs4[:, nd:], op=ALU.mult
                    )

            def b_einsum(site, c, v_tile):
                """bps[c] += sum_z1 u * v  (v broadcast over t0)."""
                prod = prodp.tile(
                    [128, T0, Z1, T1], F16, name="prB", tag=f"pr{c % 2}", bufs=1
                )
                vb = (
                    v_tile.rearrange("p (z t) -> p z t", t=T1)
                    .unsqueeze(1)
                    .broadcast_to([128, T0, Z1, T1])
                )
                split_mult(prod[:], U[:, c].rearrange("p a (z t) -> p a z t", t=T1), vb)
                z1_sum_pe(c, prod)

            def p_einsum(site, c, p_tile):
                """p = sum_t0 r * u -> p_tile [128,512] f16 (via PE identity
                accumulation in psum + Act copy)."""
                prod = prodp.tile(
                    [128, T0, 512], F16, name="prP", tag=f"pr{c % 2}", bufs=1
                )
                rb = r[c].unsqueeze(2).broadcast_to([128, T0, Z1, T1])
                split_mult(
                    prod.rearrange("p a (z t) -> p a z t", t=T1),
                    U[:, c].rearrange("p a (z t) -> p a z t", t=T1),
                    rb,
                )
                pp = ps_rt().tile([128, 512], F32, name="pp", tag=f"pp{c % 2}")
                for t0 in range(T0):
                    nc.tensor.matmul(
                        pp[:],
                        id_sb[:],
                        prod[:, t0, :],
                        start=(t0 == 0),
                        stop=(t0 == T0 - 1),
                    )
                nc.scalar.copy(p_tile[:], pp[:])

            # ---------------- conv ----------------
            pu_pair = None
            conv_psum = tc.tile_pool(name="ps_conv", bufs=1, space="PSUM")
            ps_conv = conv_psum.__enter__()
            for t0 in range(T0):
                y = build_y(t0)
                if t0 + 1 < T0:
                    load_yraw(t0 + 1)
                yv = y.rearrange("p k a b -> p k (a b)")
                for c in range(NCHUNK):
                    if c % 2 == 0:
                        pu_pair = ps_conv.tile(
                            [128, 1024], F32, name="pu", tag=f"pu{(c // 2) % 3}"
                        )
                    pu = pu_pair[:, 512 * (c % 2) : 512 * (c % 2) + 512]
                    for kw in range(3):
                        nc.tensor.matmul(
                            pu,
                            yv[:, kw, 128 * c : 128 * (c + 1)],
                            wt_sb[kw][:],
                            start=(kw == 0),
                            stop=(kw == 2),
                        )
                    if c % 2 == 1:
                        # copy pair psum -> U[:, c-1:c+1, t0, :]
                        dst = bass.AP(
                            tensor=U[:].tensor,
                            offset=(c - 1) * T0 * 512 + t0 * 512,
                            ap=[[NCHUNK * T0 * 512, 128], [T0 * 512, 2], [1, 512]],
                        )
                        src = pu_pair[:].rearrange("p (a b) -> p a b", b=512)
                        cyc = CONFIG["conv_copy"]
                        ce = cyc[(t0 * 4 + c // 2) % len(cyc)]
                        {
                            "act": nc.scalar.copy,
                            "dve": nc.vector.tensor_copy,
                            "pool": nc.gpsimd.tensor_copy,
                        }[ce](dst, src)
                if phases >= 1:
                    # p1 running sums on DVE (idle during conv)
                    if t0 == 1:
                        for c in range(NCHUNK):
                            nc.vector.tensor_tensor(
                                p1s[c][:], U[:, c, 0, :], U[:, c, 1, :], op=ALU.add
                            )
                    elif t0 >= 2:
                        for c in range(NCHUNK):
                            nc.vector.tensor_tensor(
                                p1s[c][:], p1s[c][:], U[:, c, t0, :], op=ALU.add
                            )

            conv_psum.__exit__(None, None, None)
            ps_rt_box["pool"] = ctx.enter_context(
                tc.tile_pool(name="ps_rt", bufs=1, space="PSUM")
            )

            # ---------------- routing ----------------
            # iter 1: r uniform=1/8 -> p1 = (sum_t0 u)/8; v1 = squash(p1)
            if phases >= 1:
                squash_phase(p1s, 1.0 / T0, 1, lambda c, v: b_einsum(0, c, v))

            # iters 2..3
            for it in (2, 3):
                if it > phases:
                    break
                r_phase()
                last = it == 3
                pst = []
                for c in range(NCHUNK):
                    p = small.tile(
                        [128, 512], F16, name=f"p{it}_{c}", tag=f"p{c}", bufs=1
                    )
                    p_einsum(it - 1, c, p)
                    pst.append(p)
                def out_chunk(c, v):
                    # pt partitions q=(z1l*8+t1), z1=16j+z1l; rows = hw of
                    # chunk c -> out[t1*64+z1, 128c:128c+128]
                    for j in range(4):
                        if j % 2 == 0:
                            pt = ps_rt().tile(
                                [128, 128], F32, name="pt", tag="pt", bufs=2
                            )
                        else:
                            pt = ps_rt().tile(
                                [128, 512], F32, name="pp", tag=f"pp{j // 2}"
                            )[:, 0:128]
                        nc.tensor.transpose(
                            pt[:], v[:, 128 * j : 128 * (j + 1)], idf_sb[:]
                        )
                        vtt = vout.tile(
                            [128, 128], F32, name="vtt", tag=f"vtt{(4 * c + j) % 8}"
                        )
                        nc.scalar.copy(vtt[:], pt[:])
                        dst = bass.AP(
                            tensor=out_f.tensor,
                            offset=j * 16 * HW + 128 * c,
                            ap=[[HW, 16], [64 * HW, 8], [1, 128]],
                        )
                        nc.sync.dma_start(dst, vtt[:])

                consume = (
                    (lambda c, v: b_einsum(2, c, v)) if not last else out_chunk
                )
                squash_phase(pst, 1.0, it, consume, vdt=F32 if last else F16)

    return _split_waits(nc) if split else nc


# ---------------------------------------------------------------------------
_NC = None


def _get_nc(split=True):
    global _NC
    if _NC is None:
        _NC = build_module(split)
    return _NC


def _host_prep(W, bias):
    # wt[kw][kh*32+z0, z1*8+t1] = W[t1*64+z1, z0, kh, kw]; bias in wt[1][96]
    Wr = np.asarray(W, np.float32).reshape(T1, Z1, 32, 3, 3)
    wt = np.zeros((3, K, 512), np.float32)
    # -> [kw, kh, z0, z1, t1]
    Wp = np.transpose(Wr, (4, 3, 2, 1, 0))
    wt[:, :96, :] = Wp.reshape(3, 96, 512)
    bz = np.asarray(bias, np.float32).reshape(T1, Z1).T.reshape(512)  # z1*8+t1
    wt[1, 96, :] = bz
    return wt


def _consts():
    ident = np.eye(128, dtype=np.float16)
    identf = np.eye(128, dtype=np.float32)
    ind8 = np.zeros((64, T0), np.float32)
    for p in range(64):
        ind8[p, p // T1] = 1.0
    ind8t = np.zeros((T0, 64), np.float32)
    for m in range(64):
        ind8t[m // T1, m] = 1.0
    import ml_dtypes

    return ident, identf, ind8.astype(ml_dtypes.bfloat16), ind8t.astype(ml_dtypes.bfloat16)


def _run(inputs, trace=False, **kw):
    x = np.ascontiguousarray(np.asarray(inputs["x"], np.float32))
    wt = _host_prep(inputs["W"], inputs["bias"])
    ident, identf, ind8, ind8t = _consts()
    nc = _get_nc()
    in_maps = [
        {
            "x": x[i],
            "wt": wt,
            "ident": ident,
            "identf": identf,
            "ind8": ind8,
            "ind8t": ind8t,
        }
        for i in range(8)
    ]
    res = run_bass_kernel_spmd(nc, in_maps, core_ids=list(range(8)), trace=trace, **kw)
    full = np.stack([res.results[i]["out"] for i in range(8)], axis=0)
    return full, res


def kernel(**inputs):
    full, _ = _run(inputs)
    return full


if __name__ == "__main__":
    rng = np.random.default_rng(0)
    ins = {
        "x": rng.normal(size=(8, 8, 32, 64, 64)).astype(np.float32),
        "W": (rng.normal(size=(512, 32, 3, 3)) * 0.05).astype(np.float32),
        "bias": (rng.normal(size=(512,)) * 0.01).astype(np.float32),
    }
    out = kernel(**ins)
    print(out.shape, out.dtype)
